# revision 54
# baseline (speedup 1.0000x reference)
"""Trainium2 Bass kernel for MultiHeadNodeToEdgeAttention (hypergraph node->edge).

Contract: kernel(**inputs) takes FULL unsharded inputs (numpy), returns the FULL
[E, OUT_DIM] float32 output.  Internally shards the incidence matrix along the
hyperedge axis E across 8 NeuronCores; node features and (folded) head weights
are replicated.  Softmax-over-E and min-max-normalization globals are resolved
with a single 3 KB AllGather of per-core stats.

Key algebraic folds (exact, done on host in float64):
  m[h]  = inc^T @ (nf @ W1[h])          = (inc^T @ nf) @ W1[h]
      ->  g = nf^T @ inc computed ONCE (head-independent), per-head work folds
          into 128x128 / 128x4 weight matrices applied to g.
  scores[h] = m[h] @ Wa[h] + ba[h]      -> (W1[h] @ Wa[h]) applied to g
  u~[h] = m[h] @ W2[h]                  -> (W1[h] @ W2[h]) applied to g
  b2 cancels exactly inside min-max normalization:
  (u - mn)/(mx - mn + eps) == (v - vmin)/(vmax - vmin + Z*eps)
  where v = exp(s - smax) * u~,  u = v/Z + b2.

Perf notes (TimelineSim 156 us -> 95 us/core single-shot, 52 us/iter
steady-state; hardware repeat-R slope measures ~32-37 us/iter):
  - inc/nf stream in fp16: halves the dominant HBM read (16 MB/core).
    Validated rel-err 3.2e-3 vs the 2e-2 gate.
  - nf is pre-transposed on host to [128, NCH*128] so its load is one
    contiguous DMA instead of a 4096-descriptor gather (was 5.8 us).
  - small weights ride in three packed tensors (f32 biases, f32r matmul
    weights, f16 selector/Wout) -- 3 DMAs instead of 8.
  - inc DMAs pack 2/4/8 node-chunks each (3D access patterns): fewer
    DMAs -> less serialized HWDGE dispatch overhead (625 ns each).
  - SBUF matmul operands are float32r-typed: 1 PE cycle/row instead of
    4 for plain f32 (same bits; PE-internal rounding).  p/v/rv/u tiles
    are f16 (2x DVE/ACT throughput on the post-stream tail).
  - graduated super-chunks [1024, 512, 256, 256], emitted software-
    pipelined (super k+1's matmuls queue ahead of super k's epilogue)
    so the in-order PE never head-of-line blocks; the narrow last chunk
    shortens the serial epilogue tail in front of the stats AllGather.
  - head-stat reconcile ops are emitted inside the last leaf's epilogue
    and both stacks share combined tiles: fewer serial cross-engine
    hops between the stream tail and the collective.
  - per-rep SBUF tiles are parity double-buffered and the final-stage
    PSUM rides ppb instead of the stream pool, so with repeat>1 rep
    r+1's stream fully overlaps rep r's collective + output tail
    (marginal cost/iter dropped from ~78 to ~52 us in the cost model,
    ~70 to ~35 us measured).
"""

import os

import numpy as np

import bass_rust
import concourse.bass as bass
import concourse.mybir as mybir
import concourse.tile as tile
from concourse.vector_clock import ScopedClock

# ---------------------------------------------------------------- constants
N_CORES = 8
NODE_DIM, EDGE_DIM, HIDDEN, OUT_DIM, HEADS = 128, 64, 128, 64, 4
N_NODES, N_EDGES = 4096, 16384
EPS = 1e-8
E_S = N_EDGES // N_CORES          # 2048 edges per core
NCH = N_NODES // 128              # 32 node chunks
NA = NCH // 2                     # 16 double-chunk DMA groups
ECH = 512                         # matmul moving-dim chunk
NSTACK = 2                        # head pairs stacked on 128 partitions

F32 = mybir.dt.float32
F32R = mybir.dt.float32r
_MM_DT_NAME = os.environ.get("BASS_MM_DT", "f16")
_MM_DT = {
    "f32": mybir.dt.float32,
    "f32r": mybir.dt.float32r,
    "f16": mybir.dt.float16,
    "bf16": mybir.dt.bfloat16,
}[_MM_DT_NAME]
_MM_NP = {"f32": np.float32, "f32r": np.float32,
          "f16": np.float16, "bf16": None}[_MM_DT_NAME]

# f32r-packed matmul weights (walrus requires f32r-typed producers for
# f32r matmul operands, so these live in their own f32r tensor)
_WA0 = 0                      # [128, 4]        wa_eff
_W2E0 = 4                     # [128, 2*128]    W1@W2 per stack
_SEL0 = _W2E0 + 2 * 128       # [4, 2*128]      head->stack-row selector
WPKR_COLS = _SEL0 + 2 * 128
# f32 pack: biases only
_BU0 = 0                      # [128, 2]        bias_u per stack
_BA0 = _BU0 + 2               # [4, 1]          ba_eff
_BOUT0 = _BA0 + 1             # [64, 1]         bout
WPACK_COLS = _BOUT0 + 1
# f16-packed weights (pure-f16 matmuls: p-broadcast and output)
_H_WOUT0 = 0                  # [128, 2*64]     Wout per stack
_H_SEL0 = 2 * OUT_DIM         # [4, 2*128]      selector again, f16
WPK16_COLS = _H_SEL0 + 2 * 128

# ------------------------------------------------- walrus single-wait fixes
# The pinned walrus build accepts at most ONE semaphore wait per instruction.
# Tile attaches several to the final drain and to ordinary instructions, so:
#  1) the drain keeps its waits (split afterwards like everything else),
#  2) after tracing, split every instruction with >1 waits into preceding
#     same-engine no-op carriers holding one wait each.


def _patched_drain_and_barrier(self, tick_clock, wait_clock):
    drain_inst = self.nc.sync.drain()
    wait_clock.add_sem_waits(
        drain_inst.ins, ScopedClock({None: tick_clock.global_clock})
    )
    self.nc.all_engine_barrier()
    assert self.sems is not None
    popped = self.nc._tile_sem_poison_stack.pop()
    assert popped is self._sem_poison
    self.nc.clear_and_free_semaphores(list(self.sems.allocated().values()))
    if os.environ.get("BASS_KEEP_EXIT_BARRIER", "1") == "1":
        self.nc.all_engine_barrier()


tile.TileContext._drain_and_barrier = _patched_drain_and_barrier


def _split_excess_waits(nc, maxw=1):
    for f in nc.m.functions:
        for bb in f.blocks:
            out = []
            changed = False
            for inst in bb.instructions:
                si = inst.sync_info
                waits = list(si.on_wait) if si is not None else []
                if len(waits) > maxw:
                    changed = True
                    extra, keep = waits[:-maxw], waits[-maxw:]
                    for i in range(0, len(extra), maxw):
                        nop = nc.engines[inst.engine].nop(nofuse=True)
                        ni = nop.ins
                        cb = nc.cur_bb.bb
                        assert cb.instructions[-1].name == ni.name
                        cb.instructions = cb.instructions[:-1]
                        ni.sync_info = bass_rust.SyncInfo(
                            on_wait=extra[i:i + maxw], on_update=[]
                        )
                        out.append(ni)
                    inst.sync_info = bass_rust.SyncInfo(
                        on_wait=keep, on_update=list(si.on_update)
                    )
                out.append(inst)
            if changed:
                bb.instructions = out


# ---------------------------------------------------------------- bass trace
def _build_nc(repeat=1, variant="full"):
    nc = bass.Bass("TRN2", target_bir_lowering=False, debug=False,
                   num_devices=N_CORES)

    inc = nc.dram_tensor("inc", [N_NODES, E_S], _MM_DT, kind="ExternalInput").ap()
    # host-pretransposed node features: nf_T[p, c*128 + d] = nf[c*128 + p, d]
    nf = nc.dram_tensor("nf", [128, NCH * 128], _MM_DT, kind="ExternalInput").ap()
    wpk = nc.dram_tensor("wpk", [128, WPACK_COLS], F32, kind="ExternalInput").ap()
    wpkr = nc.dram_tensor("wpkr", [128, WPKR_COLS], F32R,
                          kind="ExternalInput").ap()
    wpk16 = nc.dram_tensor("wpk16", [128, WPK16_COLS], mybir.dt.float16,
                           kind="ExternalInput").ap()
    out_T = nc.dram_tensor("out_T", [OUT_DIM, E_S], F32, kind="ExternalOutput").ap()

    # g node-chunks per DMA: node n = (a*g + b)*128 + p, chunk c = a*g + b
    inc_rp = {
        g: inc.rearrange("(a b p) e -> a p b e", b=g, p=128)
        for g in (2, 4, 8)
    }

    Exp = mybir.ActivationFunctionType.Exp
    Relu = mybir.ActivationFunctionType.Relu
    Ident = mybir.ActivationFunctionType.Identity
    Lrelu = mybir.ActivationFunctionType.Lrelu
    AX = mybir.AxisListType.X
    MUL = mybir.AluOpType.mult
    ADD = mybir.AluOpType.add
    MAX = mybir.AluOpType.max
    MIN = mybir.AluOpType.min



    with tile.TileContext(nc) as tc:
        with (
            tc.tile_pool(name="wpool", bufs=1) as wp,
            tc.tile_pool(name="incp", bufs=6) as incp,
            tc.tile_pool(name="big", bufs=1) as bg,
            tc.tile_pool(name="small", bufs=1) as sm,
            tc.tile_pool(name="pg", bufs=4, space="PSUM") as pg,
            tc.tile_pool(name="psc", bufs=1, space="PSUM") as psc,
            tc.tile_pool(name="ppb", bufs=2, space="PSUM") as ppb,
            tc.tile_pool(name="pu", bufs=1, space="PSUM") as pu,
            tc.tile_pool(name="dram", bufs=1, space="DRAM") as dram,
        ):
            # ---- resident node features + packed weights
            # first 2 chunks of nf (needed by a=0 matmuls) land first so the
            # stream can start ~immediately; the rest follows behind inc a=0.
            nf_t = wp.tile([128, NCH * 128], _MM_DT)
            nc.sync.dma_start(nf_t[:, 0:256], nf[:, 0:256])
            wpack = wp.tile([128, WPACK_COLS], F32)

            wpack16 = wp.tile([128, WPK16_COLS], mybir.dt.float16)
            wpackr = wp.tile([128, WPKR_COLS], F32R)

            wa_t = wpackr[:, _WA0:_WA0 + 4]
            ba_t = wpack[0:HEADS, _BA0:_BA0 + 1]
            bout_t = wpack[0:OUT_DIM, _BOUT0:_BOUT0 + 1]

            def w2e_s(s):
                return wpackr[:, _W2E0 + 128 * s:_W2E0 + 128 * (s + 1)]

            def bu_s(s):
                return wpack[:, _BU0 + s:_BU0 + s + 1]

            def sel_s(s):
                return wpackr[0:HEADS, _SEL0 + 128 * s:_SEL0 + 128 * (s + 1)]

            def wout16_s(s):
                return wpack16[:, _H_WOUT0 + OUT_DIM * s:
                               _H_WOUT0 + OUT_DIM * (s + 1)]

            def sel16_s(s):
                return wpack16[0:HEADS, _H_SEL0 + 128 * s:
                               _H_SEL0 + 128 * (s + 1)]

            for rep in range(repeat):
                pq = f"_{rep % 2}"
                # ---- stage B: g_T[d, e] = sum_n nf[n, d] * inc[n, e]
                # graduated super-chunk streaming: wide chunks early (amortize
                # the DMA stream), narrow chunks last (short epilogue tail).
                # Each chunk's epilogue -- g copy, scores, leaky, chunk-local
                # exp, u~, p-broadcast, v, partial extrema -- overlaps the
                # next chunk's DMA.  Chunk-local softmax frames are reconciled
                # at the end via per-chunk scales folded into the final relu.
                SUPERS = [1024, 512, 256, 256]
                # node-chunks packed per DMA: late supers use fewer, bigger
                # DMAs (fixed 625 ns HWDGE dispatch each); the first super
                # keeps small DMAs so the stream starts early.
                CPACK = [2, 4, 8, 8]
                assert sum(SUPERS) == E_S
                LEAVES = []
                off = 0
                for w in SUPERS:
                    for o in range(off, off + w, ECH):
                        LEAVES.append((o, min(ECH, off + w - o)))
                    off += w
                NLEAF = len(LEAVES)
                g_T = bg.tile([128, E_S], F32R, tag=f"gT{pq}")
                s_lk = sm.tile([HEADS, E_S], F32, tag=f"slk{pq}")
                # f16: halves ACT exp write time; p in [0,1], 2.4e-4 rel is
                # far inside the error budget (scores/g stay f32)
                p_sb = sm.tile([HEADS, E_S], mybir.dt.float16, tag=f"psb{pq}")
                msc_all = sm.tile([HEADS, NLEAF], F32, tag=f"mscall{pq}")
                nm_all = sm.tile([HEADS, NLEAF], F32, tag=f"nmall{pq}")
                z_all = sm.tile([HEADS, NLEAF], F32, tag=f"zall{pq}")
                u_sb = [bg.tile([128, E_S], mybir.dt.float16, tag=f"u{s}{pq}",
                                name=f"u{s}")
                        for s in range(NSTACK)]
                # f16 v: halves the DVE extrema-reduce time on the tail
                # critical path; extrema only need ~5e-4 relative precision
                v_sb = [bg.tile([128, E_S], mybir.dt.float16, tag=f"v{s}{pq}",
                                name=f"v{s}")
                        for s in range(NSTACK)]
                # packed extrema partials, one tile, col s*2L + t*L + ec
                # (t=0: -min(v), t=1: max(v)) so the reconcile runs one
                # multiply + one reduce over both stacks
                pmm = sm.tile([128, 2 * 2 * NLEAF], F32, tag=f"pmm{pq}")
                stats = sm.tile([128, 6], F32, tag=f"stats{pq}")
                nc.vector.memset(stats[:], 0.0)
                nsmax_l = sm.tile([HEADS, 1], F32, tag=f"nsmaxl{pq}")
                qloc2 = sm.tile([HEADS, 2 * NLEAF], F32R, tag=f"qloc2{pq}")
                zq = sm.tile([HEADS, NLEAF], F32, tag=f"zq{pq}")

                def emit_head_stats():
                    # depends only on nm_all / z_all (ready right after the
                    # last leaf's exp) -- emitted before the last leaf's
                    # extrema work so these tiny ops overlap it
                    nc.vector.tensor_scalar_mul(msc_all[:], nm_all[:], -1.0)
                    nc.vector.tensor_reduce(stats[0:HEADS, 4:5], msc_all[:],
                                            axis=AX, op=MAX)  # smax_l
                    nc.vector.tensor_reduce(nsmax_l[:], msc_all[:], axis=AX,
                                            op=MAX, negate=True)
                    # duplicated qloc so one selector matmul covers each stack
                    nc.scalar.activation(qloc2[:, 0:NLEAF], msc_all[:], Exp,
                                         bias=nsmax_l[:], scale=1.0)
                    nc.scalar.activation(qloc2[:, NLEAF:], msc_all[:], Exp,
                                         bias=nsmax_l[:], scale=1.0)
                    nc.vector.tensor_tensor(zq[:], z_all[:],
                                            qloc2[:, 0:NLEAF], op=MUL)
                    nc.vector.tensor_reduce(stats[0:HEADS, 5:6], zq[:],
                                            axis=AX, op=ADD)  # Z_l
                # flat pool slot covers the largest (cpack, width) combo
                INCTILE = max(g * w for g, w in zip(CPACK, SUPERS))

                def inc_view(t, g, w):
                    return t[:, 0:g * w].rearrange("p (b e) -> p b e", e=w)

                # prefetch the first stream chunk ahead of the bulk weight
                # loads so the PE can start at ~3 us instead of ~9.
                inc_t0 = incp.tile([128, INCTILE], _MM_DT, tag="inc")
                nc.sync.dma_start(inc_view(inc_t0, CPACK[0], SUPERS[0]),
                                  inc_rp[CPACK[0]][0][:, :, 0:SUPERS[0]])
                if rep == 0:
                    nc.sync.dma_start(wpack[:], wpk[:])
                    nc.sync.dma_start(wpackr[:], wpkr[:])
                    nc.sync.dma_start(wpack16[:], wpk16[:])
                    nc.sync.dma_start(nf_t[:, 256:1024], nf[:, 256:1024])
                    nc.sync.dma_start(nf_t[:, 1024:], nf[:, 1024:])

                def stream_super(si_):
                    w = SUPERS[si_]
                    off = sum(SUPERS[:si_])
                    leaf0 = sum((ww + ECH - 1) // ECH for ww in SUPERS[:si_])
                    nleaf = (w + ECH - 1) // ECH
                    cpk = CPACK[si_]
                    g_ps = [pg.tile([128, ECH], F32, tag="g", name="g")
                            for _ in range(nleaf)]
                    for a in range(NCH // cpk):
                        if si_ == 0 and a == 0:
                            inc_t = inc_t0
                        else:
                            inc_t = incp.tile([128, INCTILE], _MM_DT,
                                              tag="inc")
                            nc.sync.dma_start(
                                inc_view(inc_t, cpk, w),
                                inc_rp[cpk][a][:, :, off:off + w])
                        iv = inc_view(inc_t, cpk, w)
                        for b in range(cpk):
                            c = cpk * a + b
                            for h in range(nleaf):
                                lo, lw = LEAVES[leaf0 + h]
                                nc.tensor.matmul(
                                    g_ps[h][:, 0:lw],
                                    nf_t[:, c * 128:(c + 1) * 128],
                                    iv[:, b, lo - off:lo - off + lw],
                                    start=(c == 0),
                                    stop=(c == NCH - 1),
                                )
                    return g_ps

                def epilogue_super(si_, g_ps):
                    leaf0 = sum((ww + ECH - 1) // ECH for ww in SUPERS[:si_])
                    nleaf = (SUPERS[si_] + ECH - 1) // ECH
                    for h in range(nleaf):
                        ec = leaf0 + h
                        lo, lw = LEAVES[ec]
                        sl = slice(lo, lo + lw)
                        ecs = slice(ec, ec + 1)
                        # (Pool/GPSIMD cannot touch PSUM on TRN2 -- unloads
                        # must ride ACT or DVE)
                        nc.scalar.copy(g_T[:, sl], g_ps[h][:, 0:lw])
                        sc_ps = psc.tile([HEADS, ECH], F32, tag="sc")
                        nc.tensor.matmul(sc_ps[:, 0:lw], wa_t, g_T[:, sl],
                                         start=True, stop=True)
                        # leaky relu (slope .2) fused into the PSUM unload
                        nc.scalar.activation(s_lk[:, sl], sc_ps[:, 0:lw],
                                             Lrelu, bias=ba_t, scale=1.0,
                                             alpha=0.2)
                        # chunk-local softmax frame (negated max feeds exp;
                        # msc_all is recovered off the critical path later)
                        nc.vector.tensor_reduce(nm_all[:, ecs], s_lk[:, sl],
                                                axis=AX, op=MAX, negate=True)
                        nc.scalar.activation(p_sb[:, sl], s_lk[:, sl], Exp,
                                             bias=nm_all[:, ecs], scale=1.0,
                                             accum_out=z_all[:, ecs])
                        if ec == NLEAF - 1:
                            emit_head_stats()
                        for s in range(NSTACK):
                            u_ps = pu.tile([128, ECH], F32, tag="u")
                            nc.tensor.matmul(u_ps[:, 0:lw], w2e_s(s),
                                             g_T[:, sl], start=True,
                                             stop=True)
                            nc.scalar.activation(u_sb[s][:, sl],
                                                 u_ps[:, 0:lw], Ident,
                                                 bias=bu_s(s),
                                                 scale=1.0)
                            pb_ps = ppb.tile([128, ECH], F32, tag="pb")
                            nc.tensor.matmul(pb_ps[:, 0:lw], sel16_s(s),
                                             p_sb[:, sl], start=True,
                                             stop=True)
                            nc.vector.tensor_tensor(v_sb[s][:, sl],
                                                    u_sb[s][:, sl],
                                                    pb_ps[:, 0:lw], op=MUL)
                            base = 2 * NLEAF * s
                            nc.vector.tensor_reduce(
                                pmm[:, base + ec:base + ec + 1],
                                v_sb[s][:, sl], axis=AX, op=MIN,
                                negate=True)
                            nc.vector.tensor_reduce(
                                pmm[:, base + NLEAF + ec:base + NLEAF + ec + 1],
                                v_sb[s][:, sl], axis=AX, op=MAX)

                # software-pipelined emission: super k+1's g matmuls are
                # queued on the PE BEFORE super k's epilogue matmuls, so the
                # in-order PE never head-of-line blocks the stream behind
                # epilogue ops that wait on ACT/DVE
                pending = None
                for si_ in range(len(SUPERS)):
                    g_ps = stream_super(si_)
                    if pending is not None:
                        epilogue_super(*pending)
                    pending = (si_, g_ps)
                epilogue_super(*pending)

                if variant == "mm":
                    dum2 = bg.tile([OUT_DIM, E_S], F32, tag=f"osb{pq}", name="dum2")
                    nc.vector.tensor_copy(dum2[:], g_T[0:OUT_DIM, :])
                    nc.sync.dma_start(out_T[:], dum2[:])
                    continue

                # ---- reconcile chunk frames to the core-local frame.
                # (head stats were already emitted inside the last leaf)
                # stats[:, 2s+t]: t=0 -vmin_l, t=1 vmax_l  (q > 0 preserves
                # order, so max over leaves of -min*q / max*q is exact);
                # both stacks go through one multiply + one reduce
                qb_ps = ppb.tile([128, 2 * 2 * NLEAF], F32, tag="pb")
                for s in range(NSTACK):
                    nc.tensor.matmul(
                        qb_ps[:, 2 * NLEAF * s:2 * NLEAF * (s + 1)],
                        sel_s(s), qloc2[:], start=True, stop=True)
                pmc = sm.tile([128, 2 * 2 * NLEAF], F32, tag=f"pmc{pq}")
                nc.vector.tensor_tensor(pmc[:], pmm[:], qb_ps[:], op=MUL)
                nc.vector.tensor_reduce(
                    stats[:, 0:4],
                    pmc[:].rearrange("p (st l) -> p st l", l=NLEAF),
                    axis=AX, op=MAX)

                # ---- stats AllGather: [128, 6] per core -> [8, 128, 6]
                stats_all = sm.tile([128, N_CORES, 6], F32, tag=f"statsall{pq}")
                if variant == "nocoll":
                    for rr in range(N_CORES):
                        nc.vector.tensor_copy(stats_all[:, rr, :], stats[:])
                else:
                    cc_in = dram.tile([128, 6], F32)
                    cc_out = dram.tile([N_CORES, 128, 6], F32, addr_space="Shared")
                    nc.sync.dma_start(cc_in[:], stats[:])
                    nc.gpsimd.collective_compute(
                        "AllGather",
                        mybir.AluOpType.bypass,
                        ins=[cc_in[:]],
                        outs=[cc_out[:]],
                        replica_groups=[list(range(N_CORES))],
                    )
                    nc.sync.dma_start(stats_all[:],
                                      cc_out.rearrange("r p c -> p r c"))

                # ---- global reductions (tiny)
                neg_gsmax = sm.tile([HEADS, 1], F32, tag=f"ngsmax{pq}")
                nc.vector.tensor_reduce(neg_gsmax[:], stats_all[0:HEADS, :, 4],
                                        axis=AX, op=MAX, negate=True)
                c_all = sm.tile([HEADS, N_CORES], F32, tag=f"call{pq}")
                nc.scalar.activation(c_all[:], stats_all[0:HEADS, :, 4], Exp,
                                     bias=neg_gsmax[:], scale=1.0)
                c2 = sm.tile([HEADS, 2 * N_CORES], F32R, tag=f"c2{pq}")
                nc.scalar.activation(c2[:, 0:N_CORES], stats_all[0:HEADS, :, 4],
                                     Exp, bias=neg_gsmax[:], scale=1.0)
                nc.scalar.activation(c2[:, N_CORES:], stats_all[0:HEADS, :, 4],
                                     Exp, bias=neg_gsmax[:], scale=1.0)
                zc = sm.tile([HEADS, N_CORES], F32, tag=f"zc{pq}")
                nc.vector.tensor_tensor(zc[:], stats_all[0:HEADS, :, 5],
                                        c_all[:], op=MUL)
                # rhs for the per-stack broadcast matmul: [qg_all | Z_g]
                qgz = sm.tile([HEADS, NLEAF + 1], F32R, tag=f"qgz{pq}")
                nc.scalar.activation(qgz[:, 0:NLEAF], msc_all[:], Exp,
                                     bias=neg_gsmax[:], scale=1.0)
                with nc.allow_low_precision(reason="f32r is 32-bit; only PE "
                                            "rounding semantics differ"):
                    nc.vector.tensor_reduce(qgz[:, NLEAF:NLEAF + 1], zc[:],
                                            axis=AX, op=ADD)  # Z_g

                # both stacks ride combined [128, 2*K] tiles: 2 matmuls feed
                # one multiply / one reduce / one reciprocal chain
                LP1 = NLEAF + 1
                cb_ps = ppb.tile([128, 2 * 2 * N_CORES], F32, tag="pb")
                qgz_ps = pu.tile([128, 2 * LP1], F32, tag="u")
                for s in range(NSTACK):
                    nc.tensor.matmul(
                        cb_ps[:, 2 * N_CORES * s:2 * N_CORES * (s + 1)],
                        sel_s(s), c2[:], start=True, stop=True)
                    nc.tensor.matmul(qgz_ps[:, LP1 * s:LP1 * (s + 1)],
                                     sel_s(s), qgz[:],
                                     start=True, stop=True)
                gmc = sm.tile([128, 2, 2, N_CORES], F32, tag=f"gmc{pq}")
                nc.vector.tensor_tensor(
                    gmc[:],
                    stats_all[:, :, 0:4].rearrange("p r (s t) -> p s t r",
                                                   s=2),
                    cb_ps[:].rearrange("p (s t r) -> p s t r", s=2, t=2),
                    op=MUL)
                # vg2[:, 2s+0] = -vmin_g, [:, 2s+1] = vmax_g
                vg2 = sm.tile([128, 4], F32, tag=f"vg2{pq}")
                nc.vector.tensor_reduce(
                    vg2[:].rearrange("p (s t) -> p s t", s=2), gmc[:],
                    axis=AX, op=MAX)
                diff = sm.tile([128, 2], F32, tag=f"diff{pq}")
                nc.vector.tensor_add(
                    diff[:], vg2[:].rearrange("p (s t) -> p t s", t=2)[:, 1, :],
                    vg2[:].rearrange("p (s t) -> p t s", t=2)[:, 0, :])
                denom = sm.tile([128, 2], F32, tag=f"denom{pq}")
                nc.vector.scalar_tensor_tensor(
                    denom[:],
                    qgz_ps[:].rearrange("p (s k) -> p k s", k=LP1)[:, NLEAF, :],
                    EPS, diff[:], op0=MUL, op1=ADD)
                rden = sm.tile([128, 2], F32, tag=f"rden{pq}")
                nc.vector.reciprocal(rden[:], denom[:])
                # per-chunk relu scale A = qg_chunk / denom
                a_all = [sm.tile([128, NLEAF], F32, tag=f"a{s}{pq}", name=f"a{s}")
                         for s in range(NSTACK)]
                b_s = sm.tile([128, 2], F32, tag=f"bs{pq}")
                for s in range(NSTACK):
                    nc.vector.tensor_scalar(
                        a_all[s][:], qgz_ps[:, LP1 * s:LP1 * s + NLEAF],
                        rden[:, s:s + 1], None, op0=MUL)
                nc.vector.tensor_tensor(
                    b_s[:], vg2[:].rearrange("p (s t) -> p t s", t=2)[:, 0, :],
                    rden[:], op=MUL)

                # ---- normalize + relu + output matmul, chunk-pipelined
                rv = [bg.tile([128, E_S], mybir.dt.float16, tag=f"rv{s}{pq}",
                              name=f"rv{s}")
                      for s in range(NSTACK)]
                out_sb = bg.tile([OUT_DIM, E_S], F32, tag=f"osb{pq}")
                for ec in range(NLEAF):
                    lo, lw = LEAVES[ec]
                    sl = slice(lo, lo + lw)
                    # relu(a*v + b): stack 0 on ACT, stack 1 on DVE (2 ops)
                    # so the two run concurrently
                    nc.scalar.activation(rv[0][:, sl], v_sb[0][:, sl],
                                         Relu, bias=b_s[:, 0:1],
                                         scale=a_all[0][:, ec:ec + 1])
                    nc.vector.tensor_scalar(rv[1][:, sl], v_sb[1][:, sl],
                                            a_all[1][:, ec:ec + 1],
                                            b_s[:, 1:2], op0=MUL, op1=ADD)
                    nc.vector.tensor_scalar_max(rv[1][:, sl], rv[1][:, sl],
                                                0.0)
                    # out PSUM from the (now idle) 4-slot stream pool for
                    # pipelining; unload on Pool (+bout) so ACT stays on relus
                    # and DVE stays free
                    # ppb is idle in the final stage; keeping o_ps out of the
                    # stream pool (pg) lets the next rep's g accumulation
                    # start without waiting on this rep's output matmuls
                    o_ps = ppb.tile([OUT_DIM, ECH], F32, tag="pb", name="o_ps")
                    for s in range(NSTACK):
                        nc.tensor.matmul(o_ps[:, 0:lw], wout16_s(s),
                                         rv[s][:, sl],
                                         start=(s == 0), stop=(s == NSTACK - 1))
                    nc.vector.tensor_scalar(out_sb[:, sl], o_ps[:, 0:lw],
                                            bout_t, None, op0=ADD)
                    nc.sync.dma_start(out_T[:, sl], out_sb[:, sl])

    _split_excess_waits(nc)
    # strip per-instruction debug info so the NEFF cache key is independent
    # of the directory kernel.py is loaded from
    for f in nc.m.functions:
        for bb in f.blocks:
            for inst in bb.instructions:
                try:
                    inst.debug = None
                except Exception:
                    pass
    return nc


_NC_CACHE = {}


def _get_nc(repeat=1, variant="full"):
    key = ("nc", repeat, variant)
    if key not in _NC_CACHE:
        _NC_CACHE[key] = _build_nc(repeat, variant)
    return _NC_CACHE[key]


def _canonicalize_jax_source_paths():
    # HLO op metadata embeds absolute source paths; canonicalize them so the
    # neuron compile cache hits regardless of the directory kernel.py runs in.
    import jax
    try:
        jax.config.update("jax_hlo_source_file_canonicalization_regex", ".*")
    except Exception:
        pass


def _get_runner(repeat=1, variant="full"):
    """Build (once) a cached jitted SPMD executable over the 8 cores.

    Returns (fn, in_names, out_names, out_avals).  ``fn`` takes globally
    concatenated arrays (axis 0 = core) in ``in_names`` order followed by
    zero-filled output buffers, and returns concatenated outputs.
    """
    key = ("runner", repeat, variant)
    if key in _NC_CACHE:
        return _NC_CACHE[key]

    import jax
    from jax.sharding import Mesh, PartitionSpec
    from jax.experimental.shard_map import shard_map
    from concourse import bass2jax

    _canonicalize_jax_source_paths()

    nc = _get_nc(repeat, variant)
    bass2jax.install_neuronx_cc_hook()
    assert nc.dbg_addr is None
    partition_name = (nc.partition_id_tensor.name
                      if nc.partition_id_tensor else None)

    in_names, out_names, out_avals = [], [], []
    for alloc in nc.m.functions[0].allocations:
        if not isinstance(alloc, mybir.MemoryLocationSet):
            continue
        name = alloc.memorylocations[0].name
        if alloc.kind == "ExternalInput":
            if name != partition_name:
                in_names.append(name)
        elif alloc.kind == "ExternalOutput":
            out_names.append(name)
            out_avals.append(jax.core.ShapedArray(
                tuple(alloc.tensor_shape), mybir.dt.np(alloc.dtype)))
    n_params = len(in_names)
    all_names = tuple(in_names) + tuple(out_names)
    if partition_name is not None:
        all_names = all_names + (partition_name,)

    def _body(*args):
        operands = list(args)
        if partition_name is not None:
            operands.append(bass2jax.partition_id_tensor())
        outs = bass2jax._bass_exec_p.bind(
            *operands,
            out_avals=tuple(out_avals),
            in_names=all_names,
            out_names=tuple(out_names),
            lowering_input_output_aliases=(),
            sim_require_finite=True,
            sim_require_nnan=True,
            nc=nc,
        )
        return tuple(outs)

    devices = jax.devices()[:N_CORES]
    mesh = Mesh(np.asarray(devices), ("core",))
    nspecs = n_params + len(out_names)
    fn = jax.jit(shard_map(
        _body, mesh=mesh,
        in_specs=(PartitionSpec("core"),) * nspecs,
        out_specs=(PartitionSpec("core"),) * len(out_names),
        check_rep=False,
    ))
    _NC_CACHE[key] = (fn, in_names, out_names, out_avals)
    return _NC_CACHE[key]


def _run_spmd(global_in: dict, repeat=1, variant="full"):
    """global_in: name -> concatenated (8*shape0, ...) array or jax array."""
    fn, in_names, out_names, out_avals = _get_runner(repeat, variant)
    zeros = [np.zeros((N_CORES * a.shape[0], *a.shape[1:]), a.dtype)
             for a in out_avals]
    args = [global_in[n] for n in in_names] + zeros
    outs = fn(*args)
    return {n: np.asarray(o).reshape(N_CORES, *out_avals[i].shape)
            for i, (n, o) in enumerate(zip(out_names, outs))}


# ------------------------------------------------------------- host wrapper
def _fold_weights(W1, b1, Wa, ba, W2, b2, Wout, bout):
    W1d = W1.astype(np.float64)
    b1d = b1.astype(np.float64)
    Wad = Wa.astype(np.float64)
    W2d = W2.astype(np.float64)

    wa_eff = np.einsum("hdk,hk->dh", W1d, Wad).astype(np.float32)      # [128,4]
    ba_eff = (ba.astype(np.float64)
              + np.einsum("hk,hk->h", b1d, Wad)).astype(np.float32)    # [4]
    W2eff = np.einsum("hdk,hko->hdo", W1d, W2d)                        # [4,128,64]
    biasu = np.einsum("hk,hko->ho", b1d, W2d)                          # [4,64]

    wpack = np.zeros((128, WPACK_COLS), np.float32)
    wpackr = np.zeros((128, WPKR_COLS), np.float32)
    wpack16 = np.zeros((128, WPK16_COLS), np.float16)
    wpackr[:, _WA0:_WA0 + 4] = wa_eff
    for s in range(NSTACK):
        # stack s holds heads (2s, 2s+1) on rows 0:64 / 64:128
        w2e = np.concatenate([W2eff[2 * s], W2eff[2 * s + 1]], axis=1)  # [128,128]
        wpackr[:, _W2E0 + 128 * s:_W2E0 + 128 * (s + 1)] = w2e
        wpack16[:, _H_WOUT0 + OUT_DIM * s:_H_WOUT0 + OUT_DIM * (s + 1)] = \
            Wout[s * 128:(s + 1) * 128, :].astype(np.float16)
        wpack[:, _BU0 + s] = np.concatenate([biasu[2 * s], biasu[2 * s + 1]])
        wpackr[2 * s, _SEL0 + 128 * s:_SEL0 + 128 * s + 64] = 1.0
        wpackr[2 * s + 1, _SEL0 + 128 * s + 64:_SEL0 + 128 * (s + 1)] = 1.0
        wpack16[2 * s, _H_SEL0 + 128 * s:_H_SEL0 + 128 * s + 64] = 1.0
        wpack16[2 * s + 1, _H_SEL0 + 128 * s + 64:_H_SEL0 + 128 * (s + 1)] = 1.0
    wpack[0:HEADS, _BA0] = ba_eff
    wpack[0:OUT_DIM, _BOUT0] = bout.astype(np.float32)
    return {"wpk": wpack, "wpkr": wpackr, "wpk16": wpack16}


def kernel(node_features, incidence_matrix, W1, b1, Wa, ba, W2, b2, Wout, bout):
    node_features = np.asarray(node_features, np.float32)
    incidence_matrix = np.asarray(incidence_matrix, np.float32)
    weights = _fold_weights(np.asarray(W1), np.asarray(b1), np.asarray(Wa),
                            np.asarray(ba), np.asarray(W2), np.asarray(b2),
                            np.asarray(Wout), np.asarray(bout))

    if _MM_NP is np.float32:
        nf_in = node_features
        inc_full = incidence_matrix
    elif _MM_NP is None:  # bf16
        import ml_dtypes
        nf_in = node_features.astype(ml_dtypes.bfloat16)
        inc_full = incidence_matrix.astype(ml_dtypes.bfloat16)
    else:
        nf_in = node_features.astype(_MM_NP)
        inc_full = incidence_matrix.astype(_MM_NP)

    global_in = _build_global_inputs(nf_in, inc_full, weights)
    res = _run_spmd(global_in)
    out_t = res["out_T"]                      # [8, 64, 2048]
    return np.ascontiguousarray(
        out_t.transpose(0, 2, 1).reshape(N_EDGES, OUT_DIM))


def _build_global_inputs(nf_in, inc_full, weights):
    """Concatenate per-core inputs along axis 0 in one pass."""
    # core c's shard inc[:, c*E_S:(c+1)*E_S] stacked on axis 0:
    inc_g = np.ascontiguousarray(
        inc_full.reshape(N_NODES, N_CORES, E_S).transpose(1, 0, 2)
    ).reshape(N_CORES * N_NODES, E_S)
    # host pre-transpose: nf_T[p, c*128 + d] = nf[c*128 + p, d]
    nf_T = np.ascontiguousarray(
        nf_in.reshape(NCH, 128, NODE_DIM).transpose(1, 0, 2)
    ).reshape(128, NCH * NODE_DIM)
    g = {"inc": inc_g, "nf": np.concatenate([nf_T] * N_CORES, axis=0)}
    for k, v in weights.items():
        g[k] = np.concatenate([v] * N_CORES, axis=0)
    return g


# revision 59
# speedup vs baseline: 1.6495x; 1.6495x over previous
"""Trainium2 Bass kernel for MultiHeadNodeToEdgeAttention (hypergraph node->edge).

Contract: kernel(**inputs) takes FULL unsharded inputs (numpy), returns the FULL
[E, OUT_DIM] float32 output.  Internally shards the incidence matrix along the
hyperedge axis E across 8 NeuronCores; node features and (folded) head weights
are replicated.  Softmax-over-E and min-max-normalization globals are resolved
with a single 3 KB AllGather of per-core stats.

Key algebraic folds (exact, done on host in float64):
  m[h]  = inc^T @ (nf @ W1[h])          = (inc^T @ nf) @ W1[h]
      ->  g = nf^T @ inc computed ONCE (head-independent), per-head work folds
          into 128x128 / 128x4 weight matrices applied to g.
  scores[h] = m[h] @ Wa[h] + ba[h]      -> (W1[h] @ Wa[h]) applied to g
  u~[h] = m[h] @ W2[h]                  -> (W1[h] @ W2[h]) applied to g
  b2 cancels exactly inside min-max normalization:
  (u - mn)/(mx - mn + eps) == (v - vmin)/(vmax - vmin + Z*eps)
  where v = exp(s - smax) * u~,  u = v/Z + b2.

Perf notes (TimelineSim 156 us -> 95 us/core single-shot, 52 us/iter
steady-state; hardware repeat-R slope measures ~32-37 us/iter):
  - inc/nf stream in fp16: halves the dominant HBM read (16 MB/core).
    Validated rel-err 3.2e-3 vs the 2e-2 gate.
  - nf is pre-transposed on host to [128, NCH*128] so its load is one
    contiguous DMA instead of a 4096-descriptor gather (was 5.8 us).
  - small weights ride in three packed tensors (f32 biases, f32r matmul
    weights, f16 selector/Wout) -- 3 DMAs instead of 8.
  - inc DMAs pack 2/4/8 node-chunks each (3D access patterns): fewer
    DMAs -> less serialized HWDGE dispatch overhead (625 ns each).
  - SBUF matmul operands are float32r-typed: 1 PE cycle/row instead of
    4 for plain f32 (same bits; PE-internal rounding).  p/v/rv/u tiles
    are f16 (2x DVE/ACT throughput on the post-stream tail).
  - graduated super-chunks [1024, 512, 256, 256], emitted software-
    pipelined (super k+1's matmuls queue ahead of super k's epilogue)
    so the in-order PE never head-of-line blocks; the narrow last chunk
    shortens the serial epilogue tail in front of the stats AllGather.
  - head-stat reconcile ops are emitted inside the last leaf's epilogue
    and both stacks share combined tiles: fewer serial cross-engine
    hops between the stream tail and the collective.
  - per-rep SBUF tiles are parity double-buffered and the final-stage
    PSUM rides ppb instead of the stream pool, so with repeat>1 rep
    r+1's stream fully overlaps rep r's collective + output tail
    (marginal cost/iter dropped from ~78 to ~52 us in the cost model,
    ~70 to ~35 us measured).
"""

import os

import numpy as np

import bass_rust
import concourse.bass as bass
import concourse.mybir as mybir
import concourse.tile as tile
from concourse.vector_clock import ScopedClock

# ---------------------------------------------------------------- constants
N_CORES = 8
NODE_DIM, EDGE_DIM, HIDDEN, OUT_DIM, HEADS = 128, 64, 128, 64, 4
N_NODES, N_EDGES = 4096, 16384
EPS = 1e-8
E_S = N_EDGES // N_CORES          # 2048 edges per core
NCH = N_NODES // 128              # 32 node chunks
NA = NCH // 2                     # 16 double-chunk DMA groups
ECH = 512                         # matmul moving-dim chunk
NSTACK = 2                        # head pairs stacked on 128 partitions

F32 = mybir.dt.float32
F32R = mybir.dt.float32r
_MM_DT_NAME = os.environ.get("BASS_MM_DT", "f16")
_MM_DT = {
    "f32": mybir.dt.float32,
    "f32r": mybir.dt.float32r,
    "f16": mybir.dt.float16,
    "bf16": mybir.dt.bfloat16,
}[_MM_DT_NAME]
_MM_NP = {"f32": np.float32, "f32r": np.float32,
          "f16": np.float16, "bf16": None}[_MM_DT_NAME]

# f32r-packed matmul weights (walrus requires f32r-typed producers for
# f32r matmul operands, so these live in their own f32r tensor)
_WA0 = 0                      # [128, 4]        wa_eff
_W2E0 = 4                     # [128, 2*128]    W1@W2 per stack
_SEL0 = _W2E0 + 2 * 128       # [4, 2*128]      head->stack-row selector
WPKR_COLS = _SEL0 + 2 * 128
# f32 pack: biases only
_BU0 = 0                      # [128, 2]        bias_u per stack
_BA0 = _BU0 + 2               # [4, 1]          ba_eff
_BOUT0 = _BA0 + 1             # [64, 1]         bout
WPACK_COLS = _BOUT0 + 1
# f16-packed weights (pure-f16 matmuls: p-broadcast and output)
_H_WOUT0 = 0                  # [128, 2*64]     Wout per stack
_H_SEL0 = 2 * OUT_DIM         # [4, 2*128]      selector again, f16
WPK16_COLS = _H_SEL0 + 2 * 128

# ------------------------------------------------- walrus single-wait fixes
# The pinned walrus build accepts at most ONE semaphore wait per instruction.
# Tile attaches several to the final drain and to ordinary instructions, so:
#  1) the drain keeps its waits (split afterwards like everything else),
#  2) after tracing, split every instruction with >1 waits into preceding
#     same-engine no-op carriers holding one wait each.


def _patched_drain_and_barrier(self, tick_clock, wait_clock):
    drain_inst = self.nc.sync.drain()
    wait_clock.add_sem_waits(
        drain_inst.ins, ScopedClock({None: tick_clock.global_clock})
    )
    self.nc.all_engine_barrier()
    assert self.sems is not None
    popped = self.nc._tile_sem_poison_stack.pop()
    assert popped is self._sem_poison
    self.nc.clear_and_free_semaphores(list(self.sems.allocated().values()))
    if os.environ.get("BASS_KEEP_EXIT_BARRIER", "1") == "1":
        self.nc.all_engine_barrier()


tile.TileContext._drain_and_barrier = _patched_drain_and_barrier


def _split_excess_waits(nc, maxw=1):
    for f in nc.m.functions:
        for bb in f.blocks:
            out = []
            changed = False
            for inst in bb.instructions:
                si = inst.sync_info
                waits = list(si.on_wait) if si is not None else []
                if len(waits) > maxw:
                    changed = True
                    extra, keep = waits[:-maxw], waits[-maxw:]
                    for i in range(0, len(extra), maxw):
                        nop = nc.engines[inst.engine].nop(nofuse=True)
                        ni = nop.ins
                        cb = nc.cur_bb.bb
                        assert cb.instructions[-1].name == ni.name
                        cb.instructions = cb.instructions[:-1]
                        ni.sync_info = bass_rust.SyncInfo(
                            on_wait=extra[i:i + maxw], on_update=[]
                        )
                        out.append(ni)
                    inst.sync_info = bass_rust.SyncInfo(
                        on_wait=keep, on_update=list(si.on_update)
                    )
                out.append(inst)
            if changed:
                bb.instructions = out


# ---------------------------------------------------------------- bass trace
def _build_nc(repeat=1, variant="full"):
    nc = bass.Bass("TRN2", target_bir_lowering=False, debug=False,
                   num_devices=N_CORES)

    inc = nc.dram_tensor("inc", [N_NODES, E_S], _MM_DT, kind="ExternalInput").ap()
    # host-pretransposed node features: nf_T[p, c*128 + d] = nf[c*128 + p, d]
    nf = nc.dram_tensor("nf", [128, NCH * 128], _MM_DT, kind="ExternalInput").ap()
    wpk = nc.dram_tensor("wpk", [128, WPACK_COLS], F32, kind="ExternalInput").ap()
    wpkr = nc.dram_tensor("wpkr", [128, WPKR_COLS], F32R,
                          kind="ExternalInput").ap()
    wpk16 = nc.dram_tensor("wpk16", [128, WPK16_COLS], mybir.dt.float16,
                           kind="ExternalInput").ap()
    out_T = nc.dram_tensor("out_T", [OUT_DIM, E_S], mybir.dt.float16,
                       kind="ExternalOutput").ap()

    # g node-chunks per DMA: node n = (a*g + b)*128 + p, chunk c = a*g + b
    inc_rp = {
        g: inc.rearrange("(a b p) e -> a p b e", b=g, p=128)
        for g in (2, 4, 8)
    }

    Exp = mybir.ActivationFunctionType.Exp
    Relu = mybir.ActivationFunctionType.Relu
    Ident = mybir.ActivationFunctionType.Identity
    Lrelu = mybir.ActivationFunctionType.Lrelu
    AX = mybir.AxisListType.X
    MUL = mybir.AluOpType.mult
    ADD = mybir.AluOpType.add
    MAX = mybir.AluOpType.max
    MIN = mybir.AluOpType.min



    with tile.TileContext(nc) as tc:
        with (
            tc.tile_pool(name="wpool", bufs=1) as wp,
            tc.tile_pool(name="incp", bufs=6) as incp,
            tc.tile_pool(name="big", bufs=1) as bg,
            tc.tile_pool(name="small", bufs=1) as sm,
            tc.tile_pool(name="pg", bufs=4, space="PSUM") as pg,
            tc.tile_pool(name="psc", bufs=1, space="PSUM") as psc,
            tc.tile_pool(name="ppb", bufs=2, space="PSUM") as ppb,
            tc.tile_pool(name="pu", bufs=1, space="PSUM") as pu,
            tc.tile_pool(name="dram", bufs=1, space="DRAM") as dram,
        ):
            # ---- resident node features + packed weights
            # first 2 chunks of nf (needed by a=0 matmuls) land first so the
            # stream can start ~immediately; the rest follows behind inc a=0.
            nf_t = wp.tile([128, NCH * 128], _MM_DT)
            nc.sync.dma_start(nf_t[:, 0:256], nf[:, 0:256])
            wpack = wp.tile([128, WPACK_COLS], F32)

            wpack16 = wp.tile([128, WPK16_COLS], mybir.dt.float16)
            wpackr = wp.tile([128, WPKR_COLS], F32R)

            wa_t = wpackr[:, _WA0:_WA0 + 4]
            ba_t = wpack[0:HEADS, _BA0:_BA0 + 1]
            bout_t = wpack[0:OUT_DIM, _BOUT0:_BOUT0 + 1]

            def w2e_s(s):
                return wpackr[:, _W2E0 + 128 * s:_W2E0 + 128 * (s + 1)]

            def bu_s(s):
                return wpack[:, _BU0 + s:_BU0 + s + 1]

            def sel_s(s):
                return wpackr[0:HEADS, _SEL0 + 128 * s:_SEL0 + 128 * (s + 1)]

            def wout16_s(s):
                return wpack16[:, _H_WOUT0 + OUT_DIM * s:
                               _H_WOUT0 + OUT_DIM * (s + 1)]

            def sel16_s(s):
                return wpack16[0:HEADS, _H_SEL0 + 128 * s:
                               _H_SEL0 + 128 * (s + 1)]

            for rep in range(repeat):
                pq = f"_{rep % 2}"
                # ---- stage B: g_T[d, e] = sum_n nf[n, d] * inc[n, e]
                # graduated super-chunk streaming: wide chunks early (amortize
                # the DMA stream), narrow chunks last (short epilogue tail).
                # Each chunk's epilogue -- g copy, scores, leaky, chunk-local
                # exp, u~, p-broadcast, v, partial extrema -- overlaps the
                # next chunk's DMA.  Chunk-local softmax frames are reconciled
                # at the end via per-chunk scales folded into the final relu.
                SUPERS = [1024, 1024]
                # node-chunks packed per DMA: late supers use fewer, bigger
                # DMAs (fixed 625 ns HWDGE dispatch each); the first super
                # keeps small DMAs so the stream starts early.
                CPACK = [2, 4]
                assert sum(SUPERS) == E_S
                LEAVES = []
                off = 0
                for w in SUPERS:
                    for o in range(off, off + w, ECH):
                        LEAVES.append((o, min(ECH, off + w - o)))
                    off += w
                NLEAF = len(LEAVES)
                g_T = bg.tile([128, E_S], F32R, tag=f"gT{pq}")
                s_lk = sm.tile([HEADS, E_S], F32, tag=f"slk{pq}")
                # f16: halves ACT exp write time; p in [0,1], 2.4e-4 rel is
                # far inside the error budget (scores/g stay f32)
                p_sb = sm.tile([HEADS, E_S], mybir.dt.float16, tag=f"psb{pq}")
                msc_all = sm.tile([HEADS, NLEAF], F32, tag=f"mscall{pq}")
                nm_all = sm.tile([HEADS, NLEAF], F32, tag=f"nmall{pq}")
                z_all = sm.tile([HEADS, NLEAF], F32, tag=f"zall{pq}")
                u_sb = [bg.tile([128, E_S], mybir.dt.float16, tag=f"u{s}{pq}",
                                name=f"u{s}")
                        for s in range(NSTACK)]
                # f16 v: halves the DVE extrema-reduce time on the tail
                # critical path; extrema only need ~5e-4 relative precision
                v_sb = [bg.tile([128, E_S], mybir.dt.float16, tag=f"v{s}{pq}",
                                name=f"v{s}")
                        for s in range(NSTACK)]
                # packed extrema partials, one tile, col s*2L + t*L + ec
                # (t=0: -min(v), t=1: max(v)) so the reconcile runs one
                # multiply + one reduce over both stacks
                pmm = sm.tile([128, 2 * 2 * NLEAF], F32, tag=f"pmm{pq}")
                stats = sm.tile([128, 6], F32, tag=f"stats{pq}")
                nc.vector.memset(stats[:], 0.0)
                nsmax_l = sm.tile([HEADS, 1], F32, tag=f"nsmaxl{pq}")
                qloc2 = sm.tile([HEADS, 2 * NLEAF], F32R, tag=f"qloc2{pq}")
                zq = sm.tile([HEADS, NLEAF], F32, tag=f"zq{pq}")

                def emit_head_stats():
                    # depends only on nm_all / z_all (ready right after the
                    # last leaf's exp) -- emitted before the last leaf's
                    # extrema work so these tiny ops overlap it
                    nc.vector.tensor_scalar_mul(msc_all[:], nm_all[:], -1.0)
                    nc.vector.tensor_reduce(stats[0:HEADS, 4:5], msc_all[:],
                                            axis=AX, op=MAX)  # smax_l
                    nc.vector.tensor_reduce(nsmax_l[:], msc_all[:], axis=AX,
                                            op=MAX, negate=True)
                    # duplicated qloc so one selector matmul covers each stack
                    nc.scalar.activation(qloc2[:, 0:NLEAF], msc_all[:], Exp,
                                         bias=nsmax_l[:], scale=1.0)
                    nc.scalar.activation(qloc2[:, NLEAF:], msc_all[:], Exp,
                                         bias=nsmax_l[:], scale=1.0)
                    nc.vector.tensor_tensor(zq[:], z_all[:],
                                            qloc2[:, 0:NLEAF], op=MUL)
                    nc.vector.tensor_reduce(stats[0:HEADS, 5:6], zq[:],
                                            axis=AX, op=ADD)  # Z_l
                # flat pool slot covers the largest (cpack, width) combo
                INCTILE = max(g * w for g, w in zip(CPACK, SUPERS))

                def inc_view(t, g, w):
                    return t[:, 0:g * w].rearrange("p (b e) -> p b e", e=w)

                # prefetch the first stream chunk ahead of the bulk weight
                # loads so the PE can start at ~3 us instead of ~9.
                inc_t0 = incp.tile([128, INCTILE], _MM_DT, tag="inc")
                nc.sync.dma_start(inc_view(inc_t0, CPACK[0], SUPERS[0]),
                                  inc_rp[CPACK[0]][0][:, :, 0:SUPERS[0]])
                if rep == 0:
                    nc.sync.dma_start(wpack[:], wpk[:])
                    nc.sync.dma_start(wpackr[:], wpkr[:])
                    nc.sync.dma_start(wpack16[:], wpk16[:])
                    nc.sync.dma_start(nf_t[:, 256:1024], nf[:, 256:1024])
                    nc.sync.dma_start(nf_t[:, 1024:], nf[:, 1024:])

                def stream_super(si_):
                    w = SUPERS[si_]
                    off = sum(SUPERS[:si_])
                    leaf0 = sum((ww + ECH - 1) // ECH for ww in SUPERS[:si_])
                    nleaf = (w + ECH - 1) // ECH
                    cpk = CPACK[si_]
                    g_ps = [pg.tile([128, ECH], F32, tag="g", name="g")
                            for _ in range(nleaf)]
                    for a in range(NCH // cpk):
                        if si_ == 0 and a == 0:
                            inc_t = inc_t0
                        else:
                            inc_t = incp.tile([128, INCTILE], _MM_DT,
                                              tag="inc")
                            nc.sync.dma_start(
                                inc_view(inc_t, cpk, w),
                                inc_rp[cpk][a][:, :, off:off + w])
                        iv = inc_view(inc_t, cpk, w)
                        for b in range(cpk):
                            c = cpk * a + b
                            for h in range(nleaf):
                                lo, lw = LEAVES[leaf0 + h]
                                nc.tensor.matmul(
                                    g_ps[h][:, 0:lw],
                                    nf_t[:, c * 128:(c + 1) * 128],
                                    iv[:, b, lo - off:lo - off + lw],
                                    start=(c == 0),
                                    stop=(c == NCH - 1),
                                )
                    return g_ps

                def epilogue_super(si_, g_ps):
                    leaf0 = sum((ww + ECH - 1) // ECH for ww in SUPERS[:si_])
                    nleaf = (SUPERS[si_] + ECH - 1) // ECH
                    for h in range(nleaf):
                        ec = leaf0 + h
                        lo, lw = LEAVES[ec]
                        sl = slice(lo, lo + lw)
                        ecs = slice(ec, ec + 1)
                        # (Pool/GPSIMD cannot touch PSUM on TRN2 -- unloads
                        # must ride ACT or DVE)
                        nc.scalar.copy(g_T[:, sl], g_ps[h][:, 0:lw])
                        sc_ps = psc.tile([HEADS, ECH], F32, tag="sc")
                        nc.tensor.matmul(sc_ps[:, 0:lw], wa_t, g_T[:, sl],
                                         start=True, stop=True)
                        # leaky relu (slope .2) fused into the PSUM unload
                        nc.scalar.activation(s_lk[:, sl], sc_ps[:, 0:lw],
                                             Lrelu, bias=ba_t, scale=1.0,
                                             alpha=0.2)
                        # chunk-local softmax frame (negated max feeds exp;
                        # msc_all is recovered off the critical path later)
                        nc.vector.tensor_reduce(nm_all[:, ecs], s_lk[:, sl],
                                                axis=AX, op=MAX, negate=True)
                        nc.scalar.activation(p_sb[:, sl], s_lk[:, sl], Exp,
                                             bias=nm_all[:, ecs], scale=1.0,
                                             accum_out=z_all[:, ecs])
                        if ec == NLEAF - 1:
                            emit_head_stats()
                        for s in range(NSTACK):
                            u_ps = pu.tile([128, ECH], F32, tag="u")
                            nc.tensor.matmul(u_ps[:, 0:lw], w2e_s(s),
                                             g_T[:, sl], start=True,
                                             stop=True)
                            nc.scalar.activation(u_sb[s][:, sl],
                                                 u_ps[:, 0:lw], Ident,
                                                 bias=bu_s(s),
                                                 scale=1.0)
                            pb_ps = ppb.tile([128, ECH], F32, tag="pb")
                            nc.tensor.matmul(pb_ps[:, 0:lw], sel16_s(s),
                                             p_sb[:, sl], start=True,
                                             stop=True)
                            nc.vector.tensor_tensor(v_sb[s][:, sl],
                                                    u_sb[s][:, sl],
                                                    pb_ps[:, 0:lw], op=MUL)
                            base = 2 * NLEAF * s
                            nc.vector.tensor_reduce(
                                pmm[:, base + ec:base + ec + 1],
                                v_sb[s][:, sl], axis=AX, op=MIN,
                                negate=True)
                            nc.vector.tensor_reduce(
                                pmm[:, base + NLEAF + ec:base + NLEAF + ec + 1],
                                v_sb[s][:, sl], axis=AX, op=MAX)

                # software-pipelined emission: super k+1's g matmuls are
                # queued on the PE BEFORE super k's epilogue matmuls, so the
                # in-order PE never head-of-line blocks the stream behind
                # epilogue ops that wait on ACT/DVE
                pending = None
                for si_ in range(len(SUPERS)):
                    g_ps = stream_super(si_)
                    if pending is not None:
                        epilogue_super(*pending)
                    pending = (si_, g_ps)
                epilogue_super(*pending)

                if variant == "mm":
                    dum2 = bg.tile([OUT_DIM, E_S], mybir.dt.float16,
               tag=f"osb{pq}", name="dum2")
                    nc.vector.tensor_copy(dum2[:], g_T[0:OUT_DIM, :])
                    nc.sync.dma_start(out_T[:], dum2[:])
                    continue

                # ---- reconcile chunk frames to the core-local frame.
                # (head stats were already emitted inside the last leaf)
                # stats[:, 2s+t]: t=0 -vmin_l, t=1 vmax_l  (q > 0 preserves
                # order, so max over leaves of -min*q / max*q is exact);
                # both stacks go through one multiply + one reduce
                qb_ps = ppb.tile([128, 2 * 2 * NLEAF], F32, tag="pb")
                for s in range(NSTACK):
                    nc.tensor.matmul(
                        qb_ps[:, 2 * NLEAF * s:2 * NLEAF * (s + 1)],
                        sel_s(s), qloc2[:], start=True, stop=True)
                pmc = sm.tile([128, 2 * 2 * NLEAF], F32, tag=f"pmc{pq}")
                nc.vector.tensor_tensor(pmc[:], pmm[:], qb_ps[:], op=MUL)
                nc.vector.tensor_reduce(
                    stats[:, 0:4],
                    pmc[:].rearrange("p (st l) -> p st l", l=NLEAF),
                    axis=AX, op=MAX)

                # ---- stats AllGather: [128, 6] per core -> [8, 128, 6]
                stats_all = sm.tile([128, N_CORES, 6], F32, tag=f"statsall{pq}")
                if variant == "nocoll":
                    for rr in range(N_CORES):
                        nc.vector.tensor_copy(stats_all[:, rr, :], stats[:])
                else:
                    cc_in = dram.tile([128, 6], F32)
                    cc_out = dram.tile([N_CORES, 128, 6], F32, addr_space="Shared")
                    nc.sync.dma_start(cc_in[:], stats[:])
                    nc.gpsimd.collective_compute(
                        "AllGather",
                        mybir.AluOpType.bypass,
                        ins=[cc_in[:]],
                        outs=[cc_out[:]],
                        replica_groups=[list(range(N_CORES))],
                    )
                    nc.sync.dma_start(stats_all[:],
                                      cc_out.rearrange("r p c -> p r c"))

                # ---- global reductions (tiny)
                neg_gsmax = sm.tile([HEADS, 1], F32, tag=f"ngsmax{pq}")
                nc.vector.tensor_reduce(neg_gsmax[:], stats_all[0:HEADS, :, 4],
                                        axis=AX, op=MAX, negate=True)
                c_all = sm.tile([HEADS, N_CORES], F32, tag=f"call{pq}")
                nc.scalar.activation(c_all[:], stats_all[0:HEADS, :, 4], Exp,
                                     bias=neg_gsmax[:], scale=1.0)
                c2 = sm.tile([HEADS, 2 * N_CORES], F32R, tag=f"c2{pq}")
                nc.scalar.activation(c2[:, 0:N_CORES], stats_all[0:HEADS, :, 4],
                                     Exp, bias=neg_gsmax[:], scale=1.0)
                nc.scalar.activation(c2[:, N_CORES:], stats_all[0:HEADS, :, 4],
                                     Exp, bias=neg_gsmax[:], scale=1.0)
                zc = sm.tile([HEADS, N_CORES], F32, tag=f"zc{pq}")
                nc.vector.tensor_tensor(zc[:], stats_all[0:HEADS, :, 5],
                                        c_all[:], op=MUL)
                # rhs for the per-stack broadcast matmul: [qg_all | Z_g]
                # fp32r matmuls require an even moving dim (s3d3 ISA
                # restriction), so the [qg | Z_g] rhs is padded to even width
                LP1 = NLEAF + 1 + ((NLEAF + 1) % 2)
                qgz = sm.tile([HEADS, LP1], F32R, tag=f"qgz{pq}")
                if LP1 != NLEAF + 1:
                    # memset can't write f32r; park a finite dummy in the pad
                    nc.scalar.activation(qgz[:, NLEAF + 1:], neg_gsmax[:],
                                         Ident, bias=0.0, scale=1.0)
                nc.scalar.activation(qgz[:, 0:NLEAF], msc_all[:], Exp,
                                     bias=neg_gsmax[:], scale=1.0)
                with nc.allow_low_precision(reason="f32r is 32-bit; only PE "
                                            "rounding semantics differ"):
                    nc.vector.tensor_reduce(qgz[:, NLEAF:NLEAF + 1], zc[:],
                                            axis=AX, op=ADD)  # Z_g

                # both stacks ride combined [128, 2*K] tiles: 2 matmuls feed
                # one multiply / one reduce / one reciprocal chain
                cb_ps = ppb.tile([128, 2 * 2 * N_CORES], F32, tag="pb")
                qgz_ps = pu.tile([128, 2 * LP1], F32, tag="u")
                for s in range(NSTACK):
                    nc.tensor.matmul(
                        cb_ps[:, 2 * N_CORES * s:2 * N_CORES * (s + 1)],
                        sel_s(s), c2[:], start=True, stop=True)
                    nc.tensor.matmul(qgz_ps[:, LP1 * s:LP1 * (s + 1)],
                                     sel_s(s), qgz[:],
                                     start=True, stop=True)
                gmc = sm.tile([128, 2, 2, N_CORES], F32, tag=f"gmc{pq}")
                nc.vector.tensor_tensor(
                    gmc[:],
                    stats_all[:, :, 0:4].rearrange("p r (s t) -> p s t r",
                                                   s=2),
                    cb_ps[:].rearrange("p (s t r) -> p s t r", s=2, t=2),
                    op=MUL)
                # vg2[:, 2s+0] = -vmin_g, [:, 2s+1] = vmax_g
                vg2 = sm.tile([128, 4], F32, tag=f"vg2{pq}")
                nc.vector.tensor_reduce(
                    vg2[:].rearrange("p (s t) -> p s t", s=2), gmc[:],
                    axis=AX, op=MAX)
                diff = sm.tile([128, 2], F32, tag=f"diff{pq}")
                nc.vector.tensor_add(
                    diff[:], vg2[:].rearrange("p (s t) -> p t s", t=2)[:, 1, :],
                    vg2[:].rearrange("p (s t) -> p t s", t=2)[:, 0, :])
                denom = sm.tile([128, 2], F32, tag=f"denom{pq}")
                nc.vector.scalar_tensor_tensor(
                    denom[:],
                    qgz_ps[:].rearrange("p (s k) -> p k s", k=LP1)[:, NLEAF, :],
                    EPS, diff[:], op0=MUL, op1=ADD)
                rden = sm.tile([128, 2], F32, tag=f"rden{pq}")
                nc.vector.reciprocal(rden[:], denom[:])
                # per-chunk relu scale A = qg_chunk / denom
                a_all = [sm.tile([128, NLEAF], F32, tag=f"a{s}{pq}", name=f"a{s}")
                         for s in range(NSTACK)]
                b_s = sm.tile([128, 2], F32, tag=f"bs{pq}")
                for s in range(NSTACK):
                    nc.vector.tensor_scalar(
                        a_all[s][:], qgz_ps[:, LP1 * s:LP1 * s + NLEAF],
                        rden[:, s:s + 1], None, op0=MUL)
                nc.vector.tensor_tensor(
                    b_s[:], vg2[:].rearrange("p (s t) -> p t s", t=2)[:, 0, :],
                    rden[:], op=MUL)

                # ---- normalize + relu + output matmul, chunk-pipelined
                rv = [bg.tile([128, E_S], mybir.dt.float16, tag=f"rv{s}{pq}",
                              name=f"rv{s}")
                      for s in range(NSTACK)]
                out_sb = bg.tile([OUT_DIM, E_S], mybir.dt.float16,
                 tag=f"osb{pq}")
                for ec in range(NLEAF):
                    lo, lw = LEAVES[ec]
                    sl = slice(lo, lo + lw)
                    # relu(a*v + b): stack 0 on ACT, stack 1 on DVE (2 ops)
                    # so the two run concurrently
                    nc.scalar.activation(rv[0][:, sl], v_sb[0][:, sl],
                                         Relu, bias=b_s[:, 0:1],
                                         scale=a_all[0][:, ec:ec + 1])
                    nc.vector.tensor_scalar(rv[1][:, sl], v_sb[1][:, sl],
                                            a_all[1][:, ec:ec + 1],
                                            b_s[:, 1:2], op0=MUL, op1=ADD)
                    nc.vector.tensor_scalar_max(rv[1][:, sl], rv[1][:, sl],
                                                0.0)
                    # out PSUM from the (now idle) 4-slot stream pool for
                    # pipelining; unload on Pool (+bout) so ACT stays on relus
                    # and DVE stays free
                    # ppb is idle in the final stage; keeping o_ps out of the
                    # stream pool (pg) lets the next rep's g accumulation
                    # start without waiting on this rep's output matmuls
                    o_ps = ppb.tile([OUT_DIM, ECH], F32, tag="pb", name="o_ps")
                    for s in range(NSTACK):
                        nc.tensor.matmul(o_ps[:, 0:lw], wout16_s(s),
                                         rv[s][:, sl],
                                         start=(s == 0), stop=(s == NSTACK - 1))
                    nc.vector.tensor_scalar(out_sb[:, sl], o_ps[:, 0:lw],
                                            bout_t, None, op0=ADD)
                    nc.sync.dma_start(out_T[:, sl], out_sb[:, sl])

    _split_excess_waits(nc)
    # strip per-instruction debug info so the NEFF cache key is independent
    # of the directory kernel.py is loaded from
    for f in nc.m.functions:
        for bb in f.blocks:
            for inst in bb.instructions:
                try:
                    inst.debug = None
                except Exception:
                    pass
    return nc


_NC_CACHE = {}


def _get_nc(repeat=1, variant="full"):
    key = ("nc", repeat, variant)
    if key not in _NC_CACHE:
        _NC_CACHE[key] = _build_nc(repeat, variant)
    return _NC_CACHE[key]


def _canonicalize_jax_source_paths():
    # HLO op metadata embeds absolute source paths; canonicalize them so the
    # neuron compile cache hits regardless of the directory kernel.py runs in.
    import jax
    try:
        jax.config.update("jax_hlo_source_file_canonicalization_regex", ".*")
    except Exception:
        pass


def _get_runner(repeat=1, variant="full"):
    """Build (once) a cached jitted SPMD executable over the 8 cores.

    Returns (fn, in_names, out_names, out_avals).  ``fn`` takes globally
    concatenated arrays (axis 0 = core) in ``in_names`` order followed by
    zero-filled output buffers, and returns concatenated outputs.
    """
    key = ("runner", repeat, variant)
    if key in _NC_CACHE:
        return _NC_CACHE[key]

    import jax
    from jax.sharding import Mesh, PartitionSpec
    from jax.experimental.shard_map import shard_map
    from concourse import bass2jax

    _canonicalize_jax_source_paths()

    nc = _get_nc(repeat, variant)
    bass2jax.install_neuronx_cc_hook()
    assert nc.dbg_addr is None
    partition_name = (nc.partition_id_tensor.name
                      if nc.partition_id_tensor else None)

    in_names, out_names, out_avals = [], [], []
    for alloc in nc.m.functions[0].allocations:
        if not isinstance(alloc, mybir.MemoryLocationSet):
            continue
        name = alloc.memorylocations[0].name
        if alloc.kind == "ExternalInput":
            if name != partition_name:
                in_names.append(name)
        elif alloc.kind == "ExternalOutput":
            out_names.append(name)
            out_avals.append(jax.core.ShapedArray(
                tuple(alloc.tensor_shape), mybir.dt.np(alloc.dtype)))
    n_params = len(in_names)
    all_names = tuple(in_names) + tuple(out_names)
    if partition_name is not None:
        all_names = all_names + (partition_name,)

    def _body(*args):
        operands = list(args)
        if partition_name is not None:
            operands.append(bass2jax.partition_id_tensor())
        outs = bass2jax._bass_exec_p.bind(
            *operands,
            out_avals=tuple(out_avals),
            in_names=all_names,
            out_names=tuple(out_names),
            lowering_input_output_aliases=(),
            sim_require_finite=True,
            sim_require_nnan=True,
            nc=nc,
        )
        return tuple(outs)

    devices = jax.devices()[:N_CORES]
    mesh = Mesh(np.asarray(devices), ("core",))
    nspecs = n_params + len(out_names)
    fn = jax.jit(shard_map(
        _body, mesh=mesh,
        in_specs=(PartitionSpec("core"),) * nspecs,
        out_specs=(PartitionSpec("core"),) * len(out_names),
        check_rep=False,
    ))
    _NC_CACHE[key] = (fn, in_names, out_names, out_avals)
    return _NC_CACHE[key]


def _run_spmd(global_in: dict, repeat=1, variant="full"):
    """global_in: name -> concatenated (8*shape0, ...) array or jax array."""
    fn, in_names, out_names, out_avals = _get_runner(repeat, variant)
    zeros = [np.zeros((N_CORES * a.shape[0], *a.shape[1:]), a.dtype)
             for a in out_avals]
    args = [global_in[n] for n in in_names] + zeros
    outs = fn(*args)
    return {n: np.asarray(o).reshape(N_CORES, *out_avals[i].shape)
            for i, (n, o) in enumerate(zip(out_names, outs))}


# ------------------------------------------------------------- host wrapper
def _fold_weights(W1, b1, Wa, ba, W2, b2, Wout, bout):
    W1d = W1.astype(np.float64)
    b1d = b1.astype(np.float64)
    Wad = Wa.astype(np.float64)
    W2d = W2.astype(np.float64)

    wa_eff = np.einsum("hdk,hk->dh", W1d, Wad).astype(np.float32)      # [128,4]
    ba_eff = (ba.astype(np.float64)
              + np.einsum("hk,hk->h", b1d, Wad)).astype(np.float32)    # [4]
    W2eff = np.einsum("hdk,hko->hdo", W1d, W2d)                        # [4,128,64]
    biasu = np.einsum("hk,hko->ho", b1d, W2d)                          # [4,64]

    wpack = np.zeros((128, WPACK_COLS), np.float32)
    wpackr = np.zeros((128, WPKR_COLS), np.float32)
    wpack16 = np.zeros((128, WPK16_COLS), np.float16)
    wpackr[:, _WA0:_WA0 + 4] = wa_eff
    for s in range(NSTACK):
        # stack s holds heads (2s, 2s+1) on rows 0:64 / 64:128
        w2e = np.concatenate([W2eff[2 * s], W2eff[2 * s + 1]], axis=1)  # [128,128]
        wpackr[:, _W2E0 + 128 * s:_W2E0 + 128 * (s + 1)] = w2e
        wpack16[:, _H_WOUT0 + OUT_DIM * s:_H_WOUT0 + OUT_DIM * (s + 1)] = \
            Wout[s * 128:(s + 1) * 128, :].astype(np.float16)
        wpack[:, _BU0 + s] = np.concatenate([biasu[2 * s], biasu[2 * s + 1]])
        wpackr[2 * s, _SEL0 + 128 * s:_SEL0 + 128 * s + 64] = 1.0
        wpackr[2 * s + 1, _SEL0 + 128 * s + 64:_SEL0 + 128 * (s + 1)] = 1.0
        wpack16[2 * s, _H_SEL0 + 128 * s:_H_SEL0 + 128 * s + 64] = 1.0
        wpack16[2 * s + 1, _H_SEL0 + 128 * s + 64:_H_SEL0 + 128 * (s + 1)] = 1.0
    wpack[0:HEADS, _BA0] = ba_eff
    wpack[0:OUT_DIM, _BOUT0] = bout.astype(np.float32)
    return {"wpk": wpack, "wpkr": wpackr, "wpk16": wpack16}


def kernel(node_features, incidence_matrix, W1, b1, Wa, ba, W2, b2, Wout, bout):
    node_features = np.asarray(node_features, np.float32)
    incidence_matrix = np.asarray(incidence_matrix, np.float32)
    weights = _fold_weights(np.asarray(W1), np.asarray(b1), np.asarray(Wa),
                            np.asarray(ba), np.asarray(W2), np.asarray(b2),
                            np.asarray(Wout), np.asarray(bout))

    if _MM_NP is np.float32:
        nf_in = node_features
        inc_full = incidence_matrix
    elif _MM_NP is None:  # bf16
        import ml_dtypes
        nf_in = node_features.astype(ml_dtypes.bfloat16)
        inc_full = incidence_matrix.astype(ml_dtypes.bfloat16)
    else:
        nf_in = node_features.astype(_MM_NP)
        inc_full = incidence_matrix.astype(_MM_NP)

    global_in = _build_global_inputs(nf_in, inc_full, weights)
    res = _run_spmd(global_in)
    out_t = res["out_T"].astype(np.float32)   # [8, 64, 2048]
    return np.ascontiguousarray(
        out_t.transpose(0, 2, 1).reshape(N_EDGES, OUT_DIM))


def _build_global_inputs(nf_in, inc_full, weights):
    """Concatenate per-core inputs along axis 0 in one pass."""
    # core c's shard inc[:, c*E_S:(c+1)*E_S] stacked on axis 0:
    inc_g = np.ascontiguousarray(
        inc_full.reshape(N_NODES, N_CORES, E_S).transpose(1, 0, 2)
    ).reshape(N_CORES * N_NODES, E_S)
    # host pre-transpose: nf_T[p, c*128 + d] = nf[c*128 + p, d]
    nf_T = np.ascontiguousarray(
        nf_in.reshape(NCH, 128, NODE_DIM).transpose(1, 0, 2)
    ).reshape(128, NCH * NODE_DIM)
    g = {"inc": inc_g, "nf": np.concatenate([nf_T] * N_CORES, axis=0)}
    for k, v in weights.items():
        g[k] = np.concatenate([v] * N_CORES, axis=0)
    return g
